# revision 1
# baseline (speedup 1.0000x reference)
"""Causal single-head attention (B=4, T=2048, D=1024, fp32) on 8 TRN2 cores.

Sharding: core c -> (batch b = c//2, parity h = c%2). Each core computes the
output rows for query tiles qt in {2j + h : j=0..7} of its batch (zigzag
interleave of 128-row tiles, which balances the causal triangle between the
two cores sharing a batch). All 8 cores run ONE SPMD program; the parity
enters only through the data (host-gathered query stripes + mask content).

Per-core device work, all matmuls in fp32r (full-rate fp32 PE mode):
  Phase A: Q^T projection for the core's 8 query tiles (layout [e, q_local]).
  Phase B: loop over 4 key blocks of 512 keys: project K^T block and V block
           from a streamed slab of x^T, then for each live query tile:
           S = (Q^T)^T K^T in PSUM, add causal mask on diagonal blocks,
           P = exp(S/32) with row-sum accum, P^T via PE transposes,
           ctx += (P^T)^T V, accumulated in SBUF across key blocks.
  Phase C: ctx * (1/rowsum) via per-partition ACT scale, DMA out.

Host glue: transposes x/W once (DMA-efficient layouts), gathers the zigzag
query stripes, builds the parity-encoded causal mask, reassembles the output,
and adds bv at the end (softmax rows sum to 1, so ctx = P@(V+bv) = P@V + bv).
"""

import sys

sys.path.insert(0, "/opt/trn_rl_repo")

import numpy as np

import concourse.mybir as mybir
import concourse.tile as tile
from concourse import bacc
from concourse.bass_utils import run_bass_kernel_spmd
from concourse.masks import make_identity

N_CORES = 8
B, T, D = 4, 2048, 1024
P = 128
DC = D // P  # 8 contraction chunks
EC = D // P  # 8 output-feature chunks
KBW = 512  # key-block width
NKB = T // KBW  # 4 key blocks
NT = 8  # query tiles per core (of 16 per batch)
NQ = NT * P  # 1024 query rows per core
NEG = -1e30
SCALE = 1.0 / 32.0  # 1/sqrt(D)
MW = 6 * P  # mask width (512 + 2*128)

F32 = mybir.dt.float32
F32R = mybir.dt.float32r
ID = mybir.ActivationFunctionType.Identity
EXP = mybir.ActivationFunctionType.Exp


def build():
    nc = bacc.Bacc(
        "TRN2", target_bir_lowering=False, debug=False, num_devices=N_CORES
    )
    xT = nc.dram_tensor(
        "xT", [DC * NKB * P, KBW], F32R, kind="ExternalInput"
    ).ap()
    xqT = nc.dram_tensor(
        "xqT", [DC * (NQ // KBW) * P, KBW], F32R, kind="ExternalInput"
    ).ap()
    wqT = nc.dram_tensor("wqT", [D, D], F32R, kind="ExternalInput").ap()
    wkT = nc.dram_tensor("wkT", [D, D], F32R, kind="ExternalInput").ap()
    wvT = nc.dram_tensor("wvT", [D, D], F32R, kind="ExternalInput").ap()
    bq = nc.dram_tensor("bq", [D], F32, kind="ExternalInput").ap()
    bk = nc.dram_tensor("bk", [D], F32, kind="ExternalInput").ap()
    cmask = nc.dram_tensor("cmask", [P, MW], F32, kind="ExternalInput").ap()
    out = nc.dram_tensor("out", [NQ, D], F32, kind="ExternalOutput").ap()

    xT_v = xT.rearrange("(dc kb p) c -> dc kb p c", dc=DC, kb=NKB)
    xqT_v = xqT.rearrange("(dc s p) c -> dc s p c", dc=DC, s=NQ // KBW)
    wq_v = wqT.rearrange("(dc p) e -> p dc e", p=P)
    wk_v = wkT.rearrange("(dc p) e -> p dc e", p=P)
    wv_v = wvT.rearrange("(dc p) e -> p dc e", p=P)

    with tile.TileContext(nc) as tc:
        with (
            tc.tile_pool(name="const", bufs=1) as const,
            tc.tile_pool(name="w", bufs=1) as wpool,
            tc.tile_pool(name="slab", bufs=2) as slab,
            tc.tile_pool(name="big", bufs=1) as big,
            tc.tile_pool(name="p", bufs=2) as ppool,
            tc.tile_pool(name="pt", bufs=2) as ptpool,
            tc.tile_pool(name="fin", bufs=1) as fin,
            tc.tile_pool(name="psA", bufs=2, space="PSUM") as psA,
            tc.tile_pool(name="psS", bufs=2, space="PSUM") as psS,
            tc.tile_pool(name="psT", bufs=1, space="PSUM") as psT,
            tc.tile_pool(name="psC", bufs=1, space="PSUM") as psC,
        ):
            dma_rr = [0]

            def load_chunked(dst, view):
                # per-dc chunk DMAs, rotated across the DMA issue queues:
                # cuts first-use latency and spreads descriptor work
                engs = (nc.sync, nc.scalar, nc.gpsimd)
                for dc in range(DC):
                    eng = engs[dma_rr[0] % 3]
                    dma_rr[0] += 1
                    eng.dma_start(out=dst[:, dc, :], in_=view[:, dc, :])

            # ---- constants ----
            ident = const.tile([P, P], F32)
            make_identity(nc, ident)
            bigmask = const.tile([P, MW], F32)
            nc.gpsimd.dma_start(out=bigmask, in_=cmask)
            bq_sb = const.tile([P, EC], F32)
            nc.gpsimd.dma_start(out=bq_sb, in_=bq.rearrange("(c p) -> p c", p=P))
            bk_sb = const.tile([P, EC], F32)
            nc.gpsimd.dma_start(out=bk_sb, in_=bk.rearrange("(c p) -> p c", p=P))

            # persistent state
            qT_sb = big.tile([P, EC, NQ], F32R)  # Q^T, [e, local q]
            rs = big.tile([P, NT * NKB], F32)  # per (q-tile, kb) exp row-sums
            ctx_acc = [
                big.tile([P, D], F32, tag=f"ctx{j}", name=f"ctx{j}")
                for j in range(NT)
            ]

            wq_sb = wpool.tile([P, DC, D], F32R, tag="w0")

            # ---- Phase A: Q^T projection ----
            for s in range(NQ // KBW):
                sl = slab.tile([P, DC, KBW], F32R, tag="slab")
                if s == 0:
                    for dc in range(DC):
                        nc.sync.dma_start(out=sl[:, dc, :], in_=xqT_v[dc, 0])
                        nc.scalar.dma_start(
                            out=wq_sb[:, dc, :], in_=wq_v[:, dc, :]
                        )
                else:
                    for dc in range(DC):
                        eng = (nc.sync, nc.scalar, nc.gpsimd)[dma_rr[0] % 3]
                        dma_rr[0] += 1
                        eng.dma_start(out=sl[:, dc, :], in_=xqT_v[dc, s])
                for ec in range(EC):
                    ps = psA.tile([P, KBW], F32, tag="proj")
                    for dc in range(DC):
                        nc.tensor.matmul(
                            ps,
                            wq_sb[:, dc, ec * P : (ec + 1) * P],
                            sl[:, dc, :],
                            start=(dc == 0),
                            stop=(dc == DC - 1),
                        )
                    nc.scalar.activation(
                        out=qT_sb[:, ec, s * KBW : (s + 1) * KBW],
                        in_=ps,
                        func=ID,
                        bias=bq_sb[:, ec : ec + 1],
                    )

            wk_sb = wpool.tile([P, DC, D], F32R, tag="w1")
            wv_sb = None

            # ---- Phase B: key blocks ----
            sl = slab.tile([P, DC, KBW], F32R, tag="slab", name="sl0")
            # interleave the two inputs K-proj needs first
            for dc in range(DC):
                nc.sync.dma_start(out=sl[:, dc, :], in_=xT_v[dc, 0])
                nc.scalar.dma_start(out=wk_sb[:, dc, :], in_=wk_v[:, dc, :])
            for kb in range(NKB):
                kT = big.tile([P, EC, KBW], F32R, tag="kT")
                for ec in range(EC):
                    ps = psA.tile([P, KBW], F32, tag="proj")
                    for dc in range(DC):
                        nc.tensor.matmul(
                            ps,
                            wk_sb[:, dc, ec * P : (ec + 1) * P],
                            sl[:, dc, :],
                            start=(dc == 0),
                            stop=(dc == DC - 1),
                        )
                    nc.scalar.activation(
                        out=kT[:, ec, :], in_=ps, func=ID, bias=bk_sb[:, ec : ec + 1]
                    )

                if kb == 0:
                    wv_sb = wpool.tile([P, DC, D], F32R, tag="w0")  # wq slot
                    load_chunked(wv_sb, wv_v)
                v = big.tile([P, 4, D], F32R, tag="v")
                sl_next = (
                    slab.tile([P, DC, KBW], F32R, tag="slab", name=f"sl{kb + 1}")
                    if kb + 1 < NKB
                    else None
                )
                for tcc in range(4):
                    for ev in range(2):
                        ps = psA.tile([P, KBW], F32, tag="proj")
                        for dc in range(DC):
                            nc.tensor.matmul(
                                ps,
                                sl[:, dc, tcc * P : (tcc + 1) * P],
                                wv_sb[:, dc, ev * KBW : (ev + 1) * KBW],
                                start=(dc == 0),
                                stop=(dc == DC - 1),
                            )
                        nc.scalar.activation(
                            out=v[:, tcc, ev * KBW : (ev + 1) * KBW], in_=ps, func=ID
                        )
                if sl_next is not None:
                    for dc in range(DC):
                        eng = (nc.sync, nc.scalar, nc.gpsimd)[dma_rr[0] % 3]
                        dma_rr[0] += 1
                        eng.dma_start(out=sl_next[:, dc, :], in_=xT_v[dc, kb + 1])

                for j in range(NT):
                    if j // 2 < kb:  # this query tile ends before this block
                        continue
                    ntcc = 2 * (j % 2) + 2 if kb == j // 2 else 4
                    W = ntcc * P  # keys beyond W in this block are fully masked
                    ps_s = psS.tile([P, KBW], F32, tag="S")
                    for ec in range(EC):
                        nc.tensor.matmul(
                            ps_s[:, :W],
                            qT_sb[:, ec, j * P : (j + 1) * P],
                            kT[:, ec, :W],
                            start=(ec == 0),
                            stop=(ec == EC - 1),
                        )
                    if j // 2 == kb:  # diagonal block: causal mask
                        moff = (2 - 2 * (j % 2)) * P
                        nc.vector.tensor_add(
                            ps_s[:, :W], ps_s[:, :W], bigmask[:, moff : moff + W]
                        )
                    p_sb = ppool.tile([P, KBW], F32R, tag="p")
                    nc.scalar.activation(
                        out=p_sb[:, :W],
                        in_=ps_s[:, :W],
                        func=EXP,
                        scale=SCALE,
                        accum_out=rs[:, j * NKB + kb : j * NKB + kb + 1],
                    )
                    ps_t = psT.tile([P, KBW], F32, tag="pt")
                    for tcc in range(ntcc):
                        nc.tensor.matmul(
                            ps_t[:, tcc * P : (tcc + 1) * P],
                            p_sb[:, tcc * P : (tcc + 1) * P].bitcast(F32),
                            ident,
                            is_transpose=True,
                            start=True,
                            stop=True,
                        )
                    pT_sb = ptpool.tile([P, KBW], F32R, tag="pT")
                    nc.scalar.activation(
                        out=pT_sb[:, : ntcc * P], in_=ps_t[:, : ntcc * P], func=ID
                    )
                    ps_c = psC.tile([P, D], F32, tag="ctx", name="ps_c")
                    for ev in range(2):
                        sli = slice(ev * KBW, (ev + 1) * KBW)
                        for tcc in range(ntcc):
                            nc.tensor.matmul(
                                ps_c[:, sli],
                                pT_sb[:, tcc * P : (tcc + 1) * P],
                                v[:, tcc, sli],
                                start=(tcc == 0),
                                stop=(tcc == ntcc - 1),
                            )
                        if kb == 0:
                            nc.vector.tensor_copy(ctx_acc[j][:, sli], ps_c[:, sli])
                        else:
                            nc.vector.tensor_add(
                                ctx_acc[j][:, sli], ctx_acc[j][:, sli], ps_c[:, sli]
                            )

                    if kb == j // 2:  # last key block: normalize + store now
                        nkb = j // 2 + 1
                        rt = fin.tile([P, 1], F32, tag="rt", name="rt")
                        nc.vector.reduce_sum(
                            rt,
                            rs[:, j * NKB : j * NKB + nkb],
                            axis=mybir.AxisListType.X,
                        )
                        rc = fin.tile([P, 1], F32, tag="rc", name="rc")
                        nc.vector.reciprocal(rc, rt)
                        ob = fin.tile([P, D], F32, tag="ob", name="ob")
                        for ev in range(2):
                            sli = slice(ev * KBW, (ev + 1) * KBW)
                            nc.scalar.activation(
                                out=ob[:, sli], in_=ctx_acc[j][:, sli],
                                func=ID, scale=rc,
                            )
                            nc.sync.dma_start(
                                out=out[j * P : (j + 1) * P, sli], in_=ob[:, sli]
                            )

                sl = sl_next

    nc.compile()
    return nc


_cache = {}


def _get_nc():
    if "nc" not in _cache:
        _cache["nc"] = build()
    return _cache["nc"]


def _host_mask(h: int) -> np.ndarray:
    # mask[i, u] = 0 where u <= i + 256 + 128*h else NEG; sliced on-device at
    # offset (2 - 2*(j%2))*128 this yields the causal mask for qt = 2j + h.
    i = np.arange(P)[:, None]
    u = np.arange(MW)[None, :]
    return np.where(u <= i + 2 * P + h * P, 0.0, NEG).astype(np.float32)


def run(inputs, trace: bool = False):
    """Returns (output [B,T,D] fp32, BassKernelResults)."""
    nc = _get_nc()
    x = np.asarray(inputs["x"], dtype=np.float32)
    bq = np.asarray(inputs["bq"], dtype=np.float32)
    bk = np.asarray(inputs["bk"], dtype=np.float32)
    bv = np.asarray(inputs["bv"], dtype=np.float32)
    wqT = np.ascontiguousarray(np.asarray(inputs["Wq"], dtype=np.float32).T)
    wkT = np.ascontiguousarray(np.asarray(inputs["Wk"], dtype=np.float32).T)
    wvT = np.ascontiguousarray(np.asarray(inputs["Wv"], dtype=np.float32).T)
    xT = np.transpose(x, (0, 2, 1))  # [B, D, T]
    # chunk-major: [dc, kb, p, c] contiguous per (dc, kb) 256KB chunk
    xTc = np.ascontiguousarray(
        xT.reshape(B, DC, P, NKB, KBW).transpose(0, 1, 3, 2, 4)
    ).reshape(B, DC * NKB * P, KBW)

    masks = [_host_mask(0), _host_mask(1)]
    in_maps = []
    for c in range(N_CORES):
        b, h = c // 2, c % 2
        qcols = (
            np.arange(NQ) // P * 2 * P + h * P + np.arange(NQ) % P
        )  # global t of local q
        xq = xT[b][:, qcols]  # [D, NQ]
        xqc = np.ascontiguousarray(
            xq.reshape(DC, P, NQ // KBW, KBW).transpose(0, 2, 1, 3)
        ).reshape(DC * (NQ // KBW) * P, KBW)
        in_maps.append(
            {
                "xT": xTc[b],
                "xqT": xqc,
                "wqT": wqT,
                "wkT": wkT,
                "wvT": wvT,
                "bq": bq,
                "bk": bk,
                "cmask": masks[h],
            }
        )

    res = run_bass_kernel_spmd(
        nc, in_maps, core_ids=list(range(N_CORES)), trace=trace
    )

    out = np.empty((B, T, D), dtype=np.float32)
    for c in range(N_CORES):
        b, h = c // 2, c % 2
        o = res.results[c]["out"]  # [NQ, D]
        for j in range(NT):
            qt = 2 * j + h
            out[b, qt * P : (qt + 1) * P, :] = o[j * P : (j + 1) * P, :]
    out += bv  # softmax rows sum to 1, so bv folds out of the attention
    return out, res


def kernel(**inputs) -> np.ndarray:
    out, _ = run(inputs)
    return out

